# revision 2
# baseline (speedup 1.0000x reference)
"""NRI MLP decoder kernel for Trainium2 (8 NeuronCores, batch-parallel).

Strategy (per core, one batch element b):
  Grid view: all 64x64 (sender s, receiver r) pairs incl. the diagonal
  (diagonal contributions are zeroed through the rel_type grid).
  Timesteps processed in pairs (u = t parity), features x t-pair packed on
  128 SBUF partitions (partition p = u*64 + feature).

  fc1 is decomposed: A = W1a@x + b1 (receiver part), B = W1b@x (sender part)
  for all (t, node) at once (fp32r matmuls).  Per t-pair, the edge tensor
  h1pre[:, (s, r)] = A[:, r] + B[:, s] is built with one DVE add: B is
  materialized via doubling DMAs, A rides a broadcast access pattern.
  relu -> fc2 (block-diagonal weights, fp16) -> relu(+b2) on ACT from PSUM
  -> multiply by the rel_type grid (DVE) -> the per-receiver sum over s is
  fused into the first output-MLP matmul as 64 accumulating matmuls into a
  persistent PSUM region.  The rest of the output MLP runs feature-major
  over all (t, n), and the delta is added to x in fp32.
"""
import sys
import numpy as np

if "/opt/trn_rl_repo" not in sys.path:
    sys.path.insert(0, "/opt/trn_rl_repo")

import concourse.bass as bass
import concourse.tile as tile
from concourse import mybir
from concourse.bass_utils import run_bass_kernel_spmd

B, N, T, D, Kt, H = 8, 64, 50, 4, 2, 64
E = N * (N - 1)            # 4032
Q = T // 2                 # 25 t-pairs
NC = 8
G = N * N                  # 4096 grid columns per pair, s-major: col = s*64 + r
COLS = Q * N               # 1600 (q, n) columns

f32 = mybir.dt.float32
f32r = mybir.dt.float32r
f16 = mybir.dt.float16

# z3 (out-MLP layer-1 PSUM) tiling: 4 tiles x 7 pairs x 64 cols
Z3_PAIRS = 7
Z3_W = Z3_PAIRS * N        # 448 fp32 <= 512 (one PSUM bank)
Z3_TILES = 4


def _split_multi_waits(nc, max_waits=1):
    """walrus in this container rejects >1 embedded sem wait on TPB
    instructions; hoist extras into preceding same-engine NoOps."""
    for f in nc.m.functions:
        for bb in f.blocks:
            new_insts = []
            for inst in bb.instructions:
                si = inst.sync_info
                if si is not None and len(si.on_wait) > max_waits:
                    waits = list(si.on_wait)
                    keep = waits[len(waits) - max_waits:]
                    for k, w in enumerate(waits[:len(waits) - max_waits]):
                        new_insts.append(mybir.InstNoOp(
                            name=f"{inst.name}-presync-{k}", engine=inst.engine,
                            sync_info=mybir.SyncInfo(on_wait=[w], on_update=[]),
                            bass_nofuse=True))
                    inst.sync_info = mybir.SyncInfo(
                        on_wait=keep, on_update=list(si.on_update))
                new_insts.append(inst)
            bb.instructions = new_insts


def _build_fast_nc():
    nc = bass.Bass()
    dp = nc.declare_dram_parameter
    x2_d = dp("x2", [9, COLS], f32, isOutput=False)
    rtg_d = dp("rtg", [1, G], f16, isOutput=False)
    w1a_d = dp("w1a", [9, 128], f32, isOutput=False)
    w1b_d = dp("w1b", [9, 128], f32, isOutput=False)
    w2_d = dp("w2", [128, 128], f16, isOutput=False)
    o1m_d = dp("o1m", [128, 128], f16, isOutput=False)
    o1x_d = dp("o1x", [9, 128], f32, isOutput=False)
    o2m_d = dp("o2m", [128, 128], f16, isOutput=False)
    o2b_d = dp("o2b", [1, 128], f32, isOutput=False)
    o3m_d = dp("o3m", [128, 8], f16, isOutput=False)
    o3b_d = dp("o3b", [1, 8], f32, isOutput=False)
    b2c_d = dp("b2c", [128, 1], f32, isOutput=False)
    ones_d = dp("ones", [1, COLS], f32, isOutput=False)
    out_d = dp("out", [8, COLS], f32, isOutput=True)

    AO = mybir.AluOpType
    with tile.TileContext(nc) as tc:
        with tc.tile_pool(name="consts", bufs=1) as cp:
            x2 = cp.tile([9, COLS], f32r)
            nc.sync.dma_start(x2[:], x2_d[:].bitcast(f32r))
            rt = cp.tile([128, G], f16)
            nc.sync.dma_start(rt[:], rtg_d[0:1, :].partition_broadcast(128))
            w1a = cp.tile([9, 128], f32r)
            nc.sync.dma_start(w1a[:], w1a_d[:].bitcast(f32r))
            w1b = cp.tile([9, 128], f32r)
            nc.sync.dma_start(w1b[:], w1b_d[:].bitcast(f32r))
            w2 = cp.tile([128, 128], f16)
            nc.sync.dma_start(w2[:], w2_d[:])
            o1m = cp.tile([128, 128], f16)
            nc.sync.dma_start(o1m[:], o1m_d[:])
            o1x = cp.tile([9, 128], f32r)
            nc.sync.dma_start(o1x[:], o1x_d[:].bitcast(f32r))
            o2m = cp.tile([128, 128], f16)
            nc.sync.dma_start(o2m[:], o2m_d[:])
            o2b = cp.tile([1, 128], f32r)
            nc.sync.dma_start(o2b[:], o2b_d[:].bitcast(f32r))
            o3m = cp.tile([128, 8], f16)
            nc.sync.dma_start(o3m[:], o3m_d[:])
            o3b = cp.tile([1, 8], f32r)
            nc.sync.dma_start(o3b[:], o3b_d[:].bitcast(f32r))
            b2c = cp.tile([128, 1], f32)
            nc.sync.dma_start(b2c[:], b2c_d[:])
            ones = cp.tile([1, COLS], f32r)
            nc.sync.dma_start(ones[:], ones_d[:].bitcast(f32r))

            a16 = cp.tile([128, COLS], f16)
            b16 = cp.tile([128, COLS], f16)
            s1 = cp.tile([128, COLS], f16)
            s2 = cp.tile([128, COLS], f16)
            outsb = cp.tile([8, COLS], f32)

            # fc1: A = W1a@x + b1, B = W1b@x over all (q, n)
            with tc.tile_pool(name="fc1ps", bufs=2, space="PSUM") as fps:
                for c in range(4):
                    sl = slice(c * 400, (c + 1) * 400)
                    pa = fps.tile([128, 400], f32, tag="p1")
                    nc.tensor.matmul(pa[:], w1a[:], x2[:, sl], start=True, stop=True)
                    nc.scalar.copy(a16[:, sl], pa[:])
                    pb = fps.tile([128, 400], f32, tag="p1")
                    nc.tensor.matmul(pb[:], w1b[:], x2[:, sl], start=True, stop=True)
                    nc.scalar.copy(b16[:, sl], pb[:])

            with tc.tile_pool(name="z3ps", bufs=1, space="PSUM") as z3ps:
                z3t = []
                for i in range(Z3_TILES):
                    zt = z3ps.tile([128, Z3_W], f32, tag=f"z3_{i}")
                    z3t.append(zt)
                # init with x-part of out-MLP layer 1 (+ bias)
                for i in range(Z3_TILES):
                    w = min(Z3_W, COLS - i * Z3_W)
                    nc.tensor.matmul(z3t[i][:, 0:w], o1x[:],
                                     x2[:, i * Z3_W: i * Z3_W + w],
                                     start=True, stop=False, skip_group_check=True)

                with tc.tile_pool(name="work", bufs=2) as wp, \
                     tc.tile_pool(name="zps", bufs=4, space="PSUM") as zps:
                    for q in range(Q):
                        qs = slice(q * N, (q + 1) * N)
                        M = wp.tile([128, N, N], f16, tag="M")       # [p, s, r]
                        nc.sync.dma_start(M[:, :, 0:1], b16[:, qs].unsqueeze(2))
                        w = 1
                        while w < N:
                            nc.sync.dma_start(M[:, :, w:2 * w], M[:, :, 0:w])
                            w *= 2
                        h = wp.tile([128, N, N], f16, tag="h")
                        a_bc = a16[:, qs].unsqueeze(1).broadcast_to([128, N, N])
                        nc.vector.tensor_tensor(h[:], M[:], a_bc, AO.add)
                        h2 = wp.tile([128, G], f16, tag="h2")
                        nc.vector.tensor_scalar_max(
                            h2[:], h[:].rearrange("p s r -> p (s r)"), 0.0)
                        msgs = wp.tile([128, G], f16, tag="msgs")
                        for c in range(8):
                            sl = slice(c * 512, (c + 1) * 512)
                            z = zps.tile([128, 512], f32, tag="z")
                            nc.tensor.matmul(z[:], w2[:], h2[:, sl],
                                             start=True, stop=True)
                            nc.scalar.activation(msgs[:, sl], z[:],
                                                 mybir.ActivationFunctionType.Relu,
                                                 bias=b2c[:])
                        ms2 = wp.tile([128, G], f16, tag="ms2")
                        nc.vector.tensor_tensor(ms2[:], msgs[:], rt[:], AO.mult)
                        # fused rel_type-weighted sum over s: 64 accumulating
                        # matmuls into this pair's 64-col z3 region
                        zi, zoff = q // Z3_PAIRS, (q % Z3_PAIRS) * N
                        for s in range(N):
                            nc.tensor.matmul(
                                z3t[zi][:, zoff:zoff + N], o1m[:],
                                ms2[:, s * N:(s + 1) * N],
                                start=False, stop=(s == N - 1),
                                skip_group_check=True)

                # out-MLP layer 1 relu
                for i in range(Z3_TILES):
                    w = min(Z3_W, COLS - i * Z3_W)
                    nc.scalar.activation(s1[:, i * Z3_W:i * Z3_W + w],
                                         z3t[i][:, 0:w],
                                         mybir.ActivationFunctionType.Relu)

            # layers 2 and 3 + final residual add
            with tc.tile_pool(name="nodeps", bufs=4, space="PSUM") as nps:
                for c in range(4):
                    sl = slice(c * 400, (c + 1) * 400)
                    z4 = nps.tile([128, 400], f32, tag="z4")
                    nc.tensor.matmul(z4[:], o2m[:], s1[:, sl],
                                     start=True, stop=False, skip_group_check=True)
                    nc.tensor.matmul(z4[:], o2b[:], ones[:, sl],
                                     start=False, stop=True, skip_group_check=True)
                    nc.scalar.activation(s2[:, sl], z4[:],
                                         mybir.ActivationFunctionType.Relu)
                for c in range(4):
                    sl = slice(c * 400, (c + 1) * 400)
                    z5 = nps.tile([8, 400], f32, tag="z5")
                    nc.tensor.matmul(z5[:], o3m[:], s2[:, sl],
                                     start=True, stop=False, skip_group_check=True)
                    nc.tensor.matmul(z5[:], o3b[:], ones[:, sl],
                                     start=False, stop=True, skip_group_check=True)
                    nc.vector.tensor_tensor(outsb[:, sl], z5[:],
                                            x2[0:8, sl].bitcast(f32), AO.add)
            nc.sync.dma_start(out_d[:], outsb[:])

    _split_multi_waits(nc)
    return nc


def _canonical(rel_rec, rel_send):
    if rel_rec.shape != (E, N) or rel_send.shape != (E, N):
        return False
    recv_idx, send_idx = np.where(~np.eye(N, dtype=bool))
    eye = np.eye(N, dtype=rel_rec.dtype)
    return (np.array_equal(np.asarray(rel_rec), eye[recv_idx])
            and np.array_equal(np.asarray(rel_send), eye[send_idx]))


def _prep_core_inputs(x_b, rt_b, fc1_w, fc1_b, fc2_w, fc2_b,
                      out1_w, out1_b, out2_w, out2_b, out3_w, out3_b):
    """Host-side packing for one batch element. x_b: [N, T, D] fp32,
    rt_b: [E] fp32 (edge-type-1 probabilities)."""
    xt = np.ascontiguousarray(x_b.transpose(1, 0, 2))      # [T, N, D]
    # X2[u*4+d + ones row, q*64+n]
    x2 = np.empty((9, COLS), np.float32)
    x2[:8] = xt.reshape(Q, 2, N, D).transpose(1, 3, 0, 2).reshape(8, COLS)
    x2[8] = 1.0

    grid = np.zeros((N, N), np.float32)                    # [r, s]
    grid[~np.eye(N, dtype=bool)] = rt_b
    rtg = np.ascontiguousarray(grid.T).reshape(1, G).astype(np.float16)

    def blkdiag2(w):                                       # w: [in, out] -> [2in, 2out]
        z = np.zeros_like(w)
        return np.block([[w, z], [z, w]])

    W1a, W1b = fc1_w[1][:, :D], fc1_w[1][:, D:]            # [H, D]
    w1a = np.zeros((9, 128), np.float32)
    w1b = np.zeros((9, 128), np.float32)
    for u in range(2):
        for d in range(D):
            w1a[u * D + d, u * H:(u + 1) * H] = W1a[:, d]
            w1b[u * D + d, u * H:(u + 1) * H] = W1b[:, d]
    w1a[8, 0:H] = fc1_b[1]
    w1a[8, H:2 * H] = fc1_b[1]

    w2 = blkdiag2(fc2_w[1].T).astype(np.float16)           # [2H, 2H]
    o1m = blkdiag2(out1_w[:, D:].T).astype(np.float16)     # agg part
    o1x = np.zeros((9, 128), np.float32)
    for u in range(2):
        for d in range(D):
            o1x[u * D + d, u * H:(u + 1) * H] = out1_w[:, d]
    o1x[8, 0:H] = out1_b
    o1x[8, H:2 * H] = out1_b
    o2m = blkdiag2(out2_w.T).astype(np.float16)
    o2b = np.tile(out2_b, 2).reshape(1, 128).astype(np.float32)
    o3m = np.zeros((128, 8), np.float32)
    for u in range(2):
        o3m[u * H:(u + 1) * H, u * D:(u + 1) * D] = out3_w.T
    o3m = o3m.astype(np.float16)
    o3b = np.tile(out3_b, 2).reshape(1, 8).astype(np.float32)
    b2c = np.tile(fc2_b[1], 2).reshape(128, 1).astype(np.float32)
    ones = np.ones((1, COLS), np.float32)
    return {"x2": x2, "rtg": rtg, "w1a": w1a, "w1b": w1b, "w2": w2,
            "o1m": o1m, "o1x": o1x, "o2m": o2m, "o2b": o2b, "o3m": o3m,
            "o3b": o3b, "b2c": b2c, "ones": ones}


def _reference_numpy(inputs, rel_type, rel_rec, rel_send,
                     fc1_w, fc1_b, fc2_w, fc2_b,
                     out1_w, out1_b, out2_w, out2_b, out3_w, out3_b):
    """General fallback (non-canonical graphs): faithful numpy port."""
    x_in = np.swapaxes(inputs, 1, 2).astype(np.float32)
    t = x_in.shape[1]
    rt = np.broadcast_to(rel_type, (rel_type.shape[0], t) + rel_type.shape[2:])
    recv = np.einsum('en,btnd->bted', rel_rec, x_in)
    send = np.einsum('en,btnd->bted', rel_send, x_in)
    pre = np.concatenate([recv, send], -1)
    allm = np.zeros(pre.shape[:3] + (H,), np.float32)
    for i in range(1, Kt):
        h = np.maximum(np.einsum('bted,hd->bteh', pre, fc1_w[i]) + fc1_b[i], 0)
        m = np.maximum(np.einsum('bteh,oh->bteo', h, fc2_w[i]) + fc2_b[i], 0)
        allm = allm + m * rt[..., i:i + 1]
    agg = np.einsum('en,bteh->btnh', rel_rec, allm)
    aug = np.concatenate([x_in, agg], -1)
    h = np.maximum(np.einsum('btni,oi->btno', aug, out1_w) + out1_b, 0)
    h = np.maximum(np.einsum('btni,oi->btno', h, out2_w) + out2_b, 0)
    delta = np.einsum('btni,oi->btno', h, out3_w) + out3_b
    x1 = x_in + delta
    return np.swapaxes(x1[:, :t - 1], 1, 2)


def kernel(inputs, rel_type, rel_rec, rel_send,
           fc1_w, fc1_b, fc2_w, fc2_b,
           out1_w, out1_b, out2_w, out2_b, out3_w, out3_b,
           pred_steps):
    assert int(pred_steps) == 1
    args = [np.asarray(a, np.float32) for a in
            (inputs, rel_type, rel_rec, rel_send, fc1_w, fc1_b, fc2_w, fc2_b,
             out1_w, out1_b, out2_w, out2_b, out3_w, out3_b)]
    (inputs, rel_type, rel_rec, rel_send, fc1_w, fc1_b, fc2_w, fc2_b,
     out1_w, out1_b, out2_w, out2_b, out3_w, out3_b) = args

    if not _canonical(rel_rec, rel_send):
        return _reference_numpy(inputs, rel_type, rel_rec, rel_send,
                                fc1_w, fc1_b, fc2_w, fc2_b, out1_w, out1_b,
                                out2_w, out2_b, out3_w, out3_b).astype(np.float32)

    nc = _build_fast_nc()
    in_maps = []
    for b in range(B):
        in_maps.append(_prep_core_inputs(
            inputs[b], rel_type[b, 0, :, 1], fc1_w, fc1_b, fc2_w, fc2_b,
            out1_w, out1_b, out2_w, out2_b, out3_w, out3_b))
    res = run_bass_kernel_spmd(nc, in_maps, list(range(NC)))

    out = np.empty((B, N, T - 1, D), np.float32)
    for b in range(B):
        r = res.results[b]["out"]                          # [8, COLS]
        xt1 = r.reshape(2, D, Q, N).transpose(2, 0, 3, 1).reshape(T, N, D)
        out[b] = xt1[:T - 1].transpose(1, 0, 2)
    return out


# revision 3
# speedup vs baseline: 4.3347x; 4.3347x over previous
"""NRI MLP decoder kernel for Trainium2 (8 NeuronCores, batch-parallel).

Strategy (per core, one batch element b):
  Grid view: all 64x64 (sender s, receiver r) pairs incl. the diagonal
  (diagonal contributions are zeroed through the rel_type grid).
  Timesteps processed in pairs (u = t parity), features x t-pair packed on
  128 SBUF partitions (partition p = u*64 + feature).

  fc1 is decomposed: A = W1a@x + b1 (receiver part), B = W1b@x (sender part)
  for all (t, node) at once (fp32r matmuls).  Per t-pair, the edge tensor
  h1pre[:, (s, r)] = A[:, r] + B[:, s] is built with one DVE add: B is
  materialized via doubling DMAs, A rides a broadcast access pattern.
  relu -> fc2 (block-diagonal weights, fp16) -> relu(+b2) on ACT from PSUM
  -> multiply by the rel_type grid (DVE) -> the per-receiver sum over s is
  fused into the first output-MLP matmul as 64 accumulating matmuls into a
  persistent PSUM region.  The rest of the output MLP runs feature-major
  over all (t, n), and the delta is added to x in fp32.
"""
import sys
import numpy as np

if "/opt/trn_rl_repo" not in sys.path:
    sys.path.insert(0, "/opt/trn_rl_repo")

import concourse.bass as bass
import concourse.tile as tile
from concourse import mybir
from concourse.bass_utils import run_bass_kernel_spmd

B, N, T, D, Kt, H = 8, 64, 50, 4, 2, 64
E = N * (N - 1)            # 4032
Q = T // 2                 # 25 t-pairs
NC = 8
G = N * N                  # 4096 grid columns per pair, s-major: col = s*64 + r
COLS = Q * N               # 1600 (q, n) columns

f32 = mybir.dt.float32
f32r = mybir.dt.float32r
f16 = mybir.dt.float16

# z3 (out-MLP layer-1 PSUM) tiling: 4 tiles x 7 pairs x 64 cols
Z3_PAIRS = 7
Z3_W = Z3_PAIRS * N        # 448 fp32 <= 512 (one PSUM bank)
Z3_TILES = 4


def _split_multi_waits(nc, max_waits=1):
    """walrus in this container rejects >1 embedded sem wait on TPB
    instructions; hoist extras into preceding same-engine NoOps."""
    for f in nc.m.functions:
        for bb in f.blocks:
            new_insts = []
            for inst in bb.instructions:
                si = inst.sync_info
                if si is not None and len(si.on_wait) > max_waits:
                    waits = list(si.on_wait)
                    keep = waits[len(waits) - max_waits:]
                    for k, w in enumerate(waits[:len(waits) - max_waits]):
                        new_insts.append(mybir.InstNoOp(
                            name=f"{inst.name}-presync-{k}", engine=inst.engine,
                            sync_info=mybir.SyncInfo(on_wait=[w], on_update=[]),
                            bass_nofuse=True))
                    inst.sync_info = mybir.SyncInfo(
                        on_wait=keep, on_update=list(si.on_update))
                new_insts.append(inst)
            bb.instructions = new_insts


def _build_fast_nc():
    nc = bass.Bass()
    dp = nc.declare_dram_parameter
    x2_d = dp("x2", [9, COLS], f32, isOutput=False)
    rtg_d = dp("rtg", [1, G], f16, isOutput=False)
    w1a_d = dp("w1a", [9, 128], f32, isOutput=False)
    w1b_d = dp("w1b", [9, 128], f32, isOutput=False)
    w2_d = dp("w2", [128, 128], f16, isOutput=False)
    o1m_d = dp("o1m", [128, 128], f16, isOutput=False)
    o1x_d = dp("o1x", [9, 128], f32, isOutput=False)
    o2m_d = dp("o2m", [128, 128], f16, isOutput=False)
    o2b_d = dp("o2b", [1, 128], f32, isOutput=False)
    o3m_d = dp("o3m", [128, 8], f16, isOutput=False)
    o3b_d = dp("o3b", [1, 8], f32, isOutput=False)
    b2c_d = dp("b2c", [128, 1], f32, isOutput=False)
    ones_d = dp("ones", [1, COLS], f32, isOutput=False)
    out_d = dp("out", [8, COLS], f32, isOutput=True)

    AO = mybir.AluOpType
    with tile.TileContext(nc) as tc:
        with tc.tile_pool(name="consts", bufs=1) as cp:
            x2 = cp.tile([9, COLS], f32r)
            nc.sync.dma_start(x2[:], x2_d[:].bitcast(f32r))
            rt = cp.tile([128, G], f16)
            nc.sync.dma_start(rt[:], rtg_d[0:1, :].partition_broadcast(128))
            w1a = cp.tile([9, 128], f32r)
            nc.sync.dma_start(w1a[:], w1a_d[:].bitcast(f32r))
            w1b = cp.tile([9, 128], f32r)
            nc.sync.dma_start(w1b[:], w1b_d[:].bitcast(f32r))
            w2 = cp.tile([128, 128], f16)
            nc.sync.dma_start(w2[:], w2_d[:])
            o1m = cp.tile([128, 128], f16)
            nc.sync.dma_start(o1m[:], o1m_d[:])
            o1x = cp.tile([9, 128], f32r)
            nc.sync.dma_start(o1x[:], o1x_d[:].bitcast(f32r))
            o2m = cp.tile([128, 128], f16)
            nc.sync.dma_start(o2m[:], o2m_d[:])
            o2b = cp.tile([1, 128], f32r)
            nc.sync.dma_start(o2b[:], o2b_d[:].bitcast(f32r))
            o3m = cp.tile([128, 8], f16)
            nc.sync.dma_start(o3m[:], o3m_d[:])
            o3b = cp.tile([1, 8], f32r)
            nc.sync.dma_start(o3b[:], o3b_d[:].bitcast(f32r))
            b2c = cp.tile([128, 1], f32)
            nc.sync.dma_start(b2c[:], b2c_d[:])
            ones = cp.tile([1, COLS], f32r)
            nc.sync.dma_start(ones[:], ones_d[:].bitcast(f32r))

            a16 = cp.tile([128, COLS], f16)
            b16 = cp.tile([128, COLS], f16)
            s1 = cp.tile([128, COLS], f16)
            s2 = cp.tile([128, COLS], f16)
            outsb = cp.tile([8, COLS], f32)

            # fc1: A = W1a@x + b1, B = W1b@x over all (q, n)
            with tc.tile_pool(name="fc1ps", bufs=2, space="PSUM") as fps:
                for c in range(4):
                    sl = slice(c * 400, (c + 1) * 400)
                    pa = fps.tile([128, 400], f32, tag="p1")
                    nc.tensor.matmul(pa[:], w1a[:], x2[:, sl], start=True, stop=True)
                    nc.scalar.copy(a16[:, sl], pa[:])
                    pb = fps.tile([128, 400], f32, tag="p1")
                    nc.tensor.matmul(pb[:], w1b[:], x2[:, sl], start=True, stop=True)
                    nc.scalar.copy(b16[:, sl], pb[:])

            with tc.tile_pool(name="z3ps", bufs=1, space="PSUM") as z3ps:
                z3t = []
                for i in range(Z3_TILES):
                    zt = z3ps.tile([128, Z3_W], f32, tag=f"z3_{i}")
                    z3t.append(zt)
                # init with x-part of out-MLP layer 1 (+ bias)
                for i in range(Z3_TILES):
                    w = min(Z3_W, COLS - i * Z3_W)
                    nc.tensor.matmul(z3t[i][:, 0:w], o1x[:],
                                     x2[:, i * Z3_W: i * Z3_W + w],
                                     start=True, stop=False, skip_group_check=True)

                with tc.tile_pool(name="work", bufs=2) as wp, \
                     tc.tile_pool(name="zps", bufs=4, space="PSUM") as zps:
                    for q in range(Q):
                        qs = slice(q * N, (q + 1) * N)
                        # h1pre[:, (s, r)] = B[:, s] + A[:, r] via broadcast APs
                        h = wp.tile([128, N, N], f16, tag="h")
                        b_bc = b16[:, qs].unsqueeze(2).broadcast_to([128, N, N])
                        a_bc = a16[:, qs].unsqueeze(1).broadcast_to([128, N, N])
                        nc.vector.tensor_tensor(h[:], b_bc, a_bc, AO.add)
                        h2 = wp.tile([128, G], f16, tag="h2")
                        nc.vector.tensor_scalar_max(
                            h2[:], h[:].rearrange("p s r -> p (s r)"), 0.0)
                        msgs = wp.tile([128, G], f16, tag="msgs")
                        for c in range(8):
                            sl = slice(c * 512, (c + 1) * 512)
                            z = zps.tile([128, 512], f32, tag="z")
                            nc.tensor.matmul(z[:], w2[:], h2[:, sl],
                                             start=True, stop=True)
                            nc.scalar.activation(msgs[:, sl], z[:],
                                                 mybir.ActivationFunctionType.Relu,
                                                 bias=b2c[:])
                        ms2 = wp.tile([128, G], f16, tag="ms2")
                        nc.vector.tensor_tensor(ms2[:], msgs[:], rt[:], AO.mult)
                        # fused rel_type-weighted sum over s: 64 accumulating
                        # matmuls into this pair's 64-col z3 region
                        zi, zoff = q // Z3_PAIRS, (q % Z3_PAIRS) * N
                        for s in range(N):
                            nc.tensor.matmul(
                                z3t[zi][:, zoff:zoff + N], o1m[:],
                                ms2[:, s * N:(s + 1) * N],
                                start=False, stop=(s == N - 1),
                                skip_group_check=True)

                # out-MLP layer 1 relu
                for i in range(Z3_TILES):
                    w = min(Z3_W, COLS - i * Z3_W)
                    nc.scalar.activation(s1[:, i * Z3_W:i * Z3_W + w],
                                         z3t[i][:, 0:w],
                                         mybir.ActivationFunctionType.Relu)

            # layers 2 and 3 + final residual add
            with tc.tile_pool(name="nodeps", bufs=4, space="PSUM") as nps:
                for c in range(4):
                    sl = slice(c * 400, (c + 1) * 400)
                    z4 = nps.tile([128, 400], f32, tag="z4")
                    nc.tensor.matmul(z4[:], o2m[:], s1[:, sl],
                                     start=True, stop=False, skip_group_check=True)
                    nc.tensor.matmul(z4[:], o2b[:], ones[:, sl],
                                     start=False, stop=True, skip_group_check=True)
                    nc.scalar.activation(s2[:, sl], z4[:],
                                         mybir.ActivationFunctionType.Relu)
                for c in range(4):
                    sl = slice(c * 400, (c + 1) * 400)
                    z5 = nps.tile([8, 400], f32, tag="z5")
                    nc.tensor.matmul(z5[:], o3m[:], s2[:, sl],
                                     start=True, stop=False, skip_group_check=True)
                    nc.tensor.matmul(z5[:], o3b[:], ones[:, sl],
                                     start=False, stop=True, skip_group_check=True)
                    nc.vector.tensor_tensor(outsb[:, sl], z5[:],
                                            x2[0:8, sl].bitcast(f32), AO.add)
            nc.sync.dma_start(out_d[:], outsb[:])

    _split_multi_waits(nc)
    return nc


def _canonical(rel_rec, rel_send):
    if rel_rec.shape != (E, N) or rel_send.shape != (E, N):
        return False
    recv_idx, send_idx = np.where(~np.eye(N, dtype=bool))
    eye = np.eye(N, dtype=rel_rec.dtype)
    return (np.array_equal(np.asarray(rel_rec), eye[recv_idx])
            and np.array_equal(np.asarray(rel_send), eye[send_idx]))


def _prep_core_inputs(x_b, rt_b, fc1_w, fc1_b, fc2_w, fc2_b,
                      out1_w, out1_b, out2_w, out2_b, out3_w, out3_b):
    """Host-side packing for one batch element. x_b: [N, T, D] fp32,
    rt_b: [E] fp32 (edge-type-1 probabilities)."""
    xt = np.ascontiguousarray(x_b.transpose(1, 0, 2))      # [T, N, D]
    # X2[u*4+d + ones row, q*64+n]
    x2 = np.empty((9, COLS), np.float32)
    x2[:8] = xt.reshape(Q, 2, N, D).transpose(1, 3, 0, 2).reshape(8, COLS)
    x2[8] = 1.0

    grid = np.zeros((N, N), np.float32)                    # [r, s]
    grid[~np.eye(N, dtype=bool)] = rt_b
    rtg = np.ascontiguousarray(grid.T).reshape(1, G).astype(np.float16)

    def blkdiag2(w):                                       # w: [in, out] -> [2in, 2out]
        z = np.zeros_like(w)
        return np.block([[w, z], [z, w]])

    W1a, W1b = fc1_w[1][:, :D], fc1_w[1][:, D:]            # [H, D]
    w1a = np.zeros((9, 128), np.float32)
    w1b = np.zeros((9, 128), np.float32)
    for u in range(2):
        for d in range(D):
            w1a[u * D + d, u * H:(u + 1) * H] = W1a[:, d]
            w1b[u * D + d, u * H:(u + 1) * H] = W1b[:, d]
    w1a[8, 0:H] = fc1_b[1]
    w1a[8, H:2 * H] = fc1_b[1]

    w2 = blkdiag2(fc2_w[1].T).astype(np.float16)           # [2H, 2H]
    o1m = blkdiag2(out1_w[:, D:].T).astype(np.float16)     # agg part
    o1x = np.zeros((9, 128), np.float32)
    for u in range(2):
        for d in range(D):
            o1x[u * D + d, u * H:(u + 1) * H] = out1_w[:, d]
    o1x[8, 0:H] = out1_b
    o1x[8, H:2 * H] = out1_b
    o2m = blkdiag2(out2_w.T).astype(np.float16)
    o2b = np.tile(out2_b, 2).reshape(1, 128).astype(np.float32)
    o3m = np.zeros((128, 8), np.float32)
    for u in range(2):
        o3m[u * H:(u + 1) * H, u * D:(u + 1) * D] = out3_w.T
    o3m = o3m.astype(np.float16)
    o3b = np.tile(out3_b, 2).reshape(1, 8).astype(np.float32)
    b2c = np.tile(fc2_b[1], 2).reshape(128, 1).astype(np.float32)
    ones = np.ones((1, COLS), np.float32)
    return {"x2": x2, "rtg": rtg, "w1a": w1a, "w1b": w1b, "w2": w2,
            "o1m": o1m, "o1x": o1x, "o2m": o2m, "o2b": o2b, "o3m": o3m,
            "o3b": o3b, "b2c": b2c, "ones": ones}


def _reference_numpy(inputs, rel_type, rel_rec, rel_send,
                     fc1_w, fc1_b, fc2_w, fc2_b,
                     out1_w, out1_b, out2_w, out2_b, out3_w, out3_b):
    """General fallback (non-canonical graphs): faithful numpy port."""
    x_in = np.swapaxes(inputs, 1, 2).astype(np.float32)
    t = x_in.shape[1]
    rt = np.broadcast_to(rel_type, (rel_type.shape[0], t) + rel_type.shape[2:])
    recv = np.einsum('en,btnd->bted', rel_rec, x_in)
    send = np.einsum('en,btnd->bted', rel_send, x_in)
    pre = np.concatenate([recv, send], -1)
    allm = np.zeros(pre.shape[:3] + (H,), np.float32)
    for i in range(1, Kt):
        h = np.maximum(np.einsum('bted,hd->bteh', pre, fc1_w[i]) + fc1_b[i], 0)
        m = np.maximum(np.einsum('bteh,oh->bteo', h, fc2_w[i]) + fc2_b[i], 0)
        allm = allm + m * rt[..., i:i + 1]
    agg = np.einsum('en,bteh->btnh', rel_rec, allm)
    aug = np.concatenate([x_in, agg], -1)
    h = np.maximum(np.einsum('btni,oi->btno', aug, out1_w) + out1_b, 0)
    h = np.maximum(np.einsum('btni,oi->btno', h, out2_w) + out2_b, 0)
    delta = np.einsum('btni,oi->btno', h, out3_w) + out3_b
    x1 = x_in + delta
    return np.swapaxes(x1[:, :t - 1], 1, 2)


def kernel(inputs, rel_type, rel_rec, rel_send,
           fc1_w, fc1_b, fc2_w, fc2_b,
           out1_w, out1_b, out2_w, out2_b, out3_w, out3_b,
           pred_steps):
    assert int(pred_steps) == 1
    args = [np.asarray(a, np.float32) for a in
            (inputs, rel_type, rel_rec, rel_send, fc1_w, fc1_b, fc2_w, fc2_b,
             out1_w, out1_b, out2_w, out2_b, out3_w, out3_b)]
    (inputs, rel_type, rel_rec, rel_send, fc1_w, fc1_b, fc2_w, fc2_b,
     out1_w, out1_b, out2_w, out2_b, out3_w, out3_b) = args

    if not _canonical(rel_rec, rel_send):
        return _reference_numpy(inputs, rel_type, rel_rec, rel_send,
                                fc1_w, fc1_b, fc2_w, fc2_b, out1_w, out1_b,
                                out2_w, out2_b, out3_w, out3_b).astype(np.float32)

    nc = _build_fast_nc()
    in_maps = []
    for b in range(B):
        in_maps.append(_prep_core_inputs(
            inputs[b], rel_type[b, 0, :, 1], fc1_w, fc1_b, fc2_w, fc2_b,
            out1_w, out1_b, out2_w, out2_b, out3_w, out3_b))
    res = run_bass_kernel_spmd(nc, in_maps, list(range(NC)))

    out = np.empty((B, N, T - 1, D), np.float32)
    for b in range(B):
        r = res.results[b]["out"]                          # [8, COLS]
        xt1 = r.reshape(2, D, Q, N).transpose(2, 0, 3, 1).reshape(T, N, D)
        out[b] = xt1[:T - 1].transpose(1, 0, 2)
    return out
